# revision 23
# baseline (speedup 1.0000x reference)
"""Trainium2 Bass kernel for MultiHeadSelfAttention with RoPE.

Problem: x[2, 2048, 1024] @ W_qkv[1024, 3072] -> rope(q,k) -> softmax(q k^T/8) v
         -> out @ W_out[1024, 1024].

Sharding (8 cores): batch (2-way) x head-group (4-way, 4 heads each).
Each core computes a partial output [2048, 1024] = attnout_heads @ W_out_rows;
host sums the 4 head-group partials per batch.

v5 design:
  - scores: two concurrent K=64 matmuls via PE row-tiling (head A on array
    rows 0:63, head B on rows 64:127).
  - attention interleaves the TWO head pairs at the sk level: while pair g's
    exp (ScalarE, 1.15us) runs, the PE computes pair g^1's scores/attn@v, so
    the scores->exp->scores WAR chain always has a full exp of slack and the
    PE stays dense (HAM stays at 2.4 GHz).
  - exp: one ACT per (pair, sk) over [128, 1024] (both heads); every 4th sk
    uses a Schraudolph fast-exp on DVE+gpsimd instead to keep ScalarE below
    the PE pace.
  - attn@v in bf16 (fp8 fails the accuracy budget: attention output is a
    weighted average, so weight/value quantization error does NOT average
    out), M=65 with a ones-column so PSUM row 64 accumulates the softmax
    denominator for free.
  - PSUM: scores ping-pong 2xpair regions (4 banks) + 4 attn@v accumulators
    (4 banks). Projections run upfront in their own pool epoch; output
    projection runs at the end.
"""

import sys

if "/opt/trn_rl_repo" not in sys.path:
    sys.path.insert(0, "/opt/trn_rl_repo")

import numpy as np

B, S, E = 2, 2048, 1024
ATT = 1024
H = 16
D = 64
HG = 4            # head groups (cores per batch)
HPG = H // HG     # heads per core = 4
PAIRS = HPG // 2  # head pairs per core = 2
ROPE_THETA = 10000.0
N_CORES = 8

SQ = 512          # sq chunk
N_CH = S // SQ    # 4 chunks
N_SK = S // 128   # 16 sk tiles
EK = E // 128     # 8 contraction tiles over embedding dim

# -ln(64): numerator and denominator share the shift so softmax is unchanged.
EXP_BIAS = -4.1588830833596715
# Schraudolph fast-exp: i32 = int(A*s + B); bitcast(i32) ~ exp(0.125*s +
# EXP_BIAS) * (1 +- 1.8% rms). C=482804 calibrated on hardware (mini_test2)
# to zero the mean log error so Schraudolph'd softmax weights are unbiased
# vs the ACT-exp'd ones.
SCH_A = 0.125 * 12102203.161561485
SCH_B = 1065353216.0 - 50331648.0 - 482804.0

_BUILT = {}
DBG = False


def _build_program():
    import concourse.bacc as bacc
    import concourse.tile as tile
    import concourse.mybir as mybir

    f32 = mybir.dt.float32
    bf16 = mybir.dt.bfloat16
    i32 = mybir.dt.int32
    AF = mybir.ActivationFunctionType
    ALU = mybir.AluOpType

    nc = bacc.Bacc(
        "TRN2",
        target_bir_lowering=False,
        debug=False,
        enable_asserts=False,
        num_devices=N_CORES,
    )

    xT = nc.dram_tensor("xT", [E, S], bf16, kind="ExternalInput").ap()
    w_qk = nc.dram_tensor("w_qk", [E, 2 * HPG * D], bf16, kind="ExternalInput").ap()
    w_v = nc.dram_tensor("w_v", [E, HPG * D], bf16, kind="ExternalInput").ap()
    w_o = nc.dram_tensor("w_o", [HPG * D, E], bf16, kind="ExternalInput").ap()
    cos_t = nc.dram_tensor("cos_t", [128, S], f32, kind="ExternalInput").ap()
    sin_t = nc.dram_tensor("sin_t", [128, S], f32, kind="ExternalInput").ap()
    mswap = nc.dram_tensor("mswap", [128, 128], bf16, kind="ExternalInput").ap()
    out = nc.dram_tensor("out", [S, E], bf16, kind="ExternalOutput").ap()

    with tile.TileContext(nc) as tc:
        with (
            tc.tile_pool(name="const", bufs=1) as constp,
            tc.tile_pool(name="xt", bufs=1) as xtp,
            tc.tile_pool(name="wqk", bufs=1) as wqkp,
            tc.tile_pool(name="trig", bufs=1) as trigp,
            tc.tile_pool(name="qkT", bufs=1) as qkTp,
            tc.tile_pool(name="vc", bufs=1) as vp,
            tc.tile_pool(name="attnout", bufs=1) as aop,
            tc.tile_pool(name="wo", bufs=1) as wop,
            tc.tile_pool(name="ropes", bufs=2) as ropep,
            tc.tile_pool(name="exps", bufs=1) as expp,
            tc.tile_pool(name="norm", bufs=2) as rcp,
            tc.tile_pool(name="osb", bufs=3) as osbp,
        ):
            # ---------------- static tiles ----------------
            msw_sb = constp.tile([128, 128], bf16, tag="msw")
            onesrow = constp.tile([65, 64], bf16, tag="onesrow")
            bias_t = constp.tile([128, 1], f32, tag="bias")
            nc.gpsimd.memset(onesrow[64:65, :], 1.0)
            nc.gpsimd.memset(bias_t[:], EXP_BIAS)

            qT = [qkTp.tile([128, S], bf16, tag=f"qT{g}", name=f"qT{g}") for g in range(PAIRS)]
            kT = [qkTp.tile([128, S], bf16, tag=f"kT{g}", name=f"kT{g}") for g in range(PAIRS)]
            # v in bf16: [ki, st, head, 65]; col 64 = ones (denominator aug)
            v_c = vp.tile([128, N_SK, HPG, 65], bf16, tag="vc")
            nc.gpsimd.memset(v_c[:, :, :, 64:65], 1.0)
            att_o = [aop.tile([128, S], bf16, tag=f"ao{g}", name=f"ao{g}") for g in range(PAIRS)]
            wo_sb = [wop.tile([128, E], bf16, tag=f"wo{g}", name=f"wo{g}") for g in range(PAIRS)]

            cos_sb = trigp.tile([128, S], f32, tag="cos")
            sin_sb = trigp.tile([128, S], f32, tag="sin")
            wqk_all = wqkp.tile([128, EK, 2 * HPG * D], bf16, tag="wqk")
            wv_all = wqkp.tile([128, EK, HPG * D], bf16, tag="wv")
            xt_all = xtp.tile([128, EK, S], bf16, tag="xt")

            # ---------------- DMA (consumption order) ----------------
            nc.sync.dma_start(msw_sb[:], mswap[:])
            wqk_d = w_qk.rearrange("(ek p) c -> p ek c", p=128)
            xt_d = xT.rearrange("(ek p) s -> p ek s", p=128)
            nc.sync.dma_start(wqk_all[:], wqk_d)
            nc.sync.dma_start(xt_all[:, :, 0:512], xt_d[:, :, 0:512])
            nc.sync.dma_start(cos_sb[:, 0:512], cos_t[:, 0:512])
            nc.sync.dma_start(sin_sb[:, 0:512], sin_t[:, 0:512])
            for c in range(1, 4):
                csl = slice(512 * c, 512 * (c + 1))
                nc.sync.dma_start(xt_all[:, :, csl], xt_d[:, :, csl])
                nc.sync.dma_start(cos_sb[:, csl], cos_t[:, csl])
                nc.sync.dma_start(sin_sb[:, csl], sin_t[:, csl])
            nc.sync.dma_start(wv_all[:], w_v.rearrange("(ek p) c -> p ek c", p=128))
            for g in range(PAIRS):
                nc.sync.dma_start(wo_sb[g][:], w_o[128 * g : 128 * (g + 1), :])

            wqk_sb = [wqk_all[:, e, :] for e in range(EK)]
            xt_sb = [xt_all[:, e, :] for e in range(EK)]
            wv_sb = [wv_all[:, e, :] for e in range(EK)]

            if DBG:
                d_qT = nc.dram_tensor("d_qT", [128, S], bf16, kind="ExternalOutput").ap()
                d_kT = nc.dram_tensor("d_kT", [128, S], bf16, kind="ExternalOutput").ap()

            def heat(n):
                # Standalone LDWEIGHTS as a PE "heater": keeps the PE busy
                # through producer-chain waits so the HAM never re-throttles
                # the clock to 1.2 GHz. Harmless: every real matmul self-loads
                # its own weights.
                for _ in range(n):
                    nc.tensor.ldweights(wqk_all[:, 0, 0:128])

            # ---------------- phase 1: projections + rope (own PSUM epoch) --
            with tc.tile_pool(name="pps", bufs=1, space="PSUM") as pjp:
                rope_pend = []

                def rope_tail():
                    (dest, sl, raw) = rope_pend.pop(0)
                    rp = pjp.tile([128, 512], f32, tag="rot", bufs=2, name="rp")
                    nc.tensor.matmul(rp[:], msw_sb[:], raw[:], start=True, stop=True)
                    t2 = ropep.tile([128, 512], f32, tag="t2")
                    nc.vector.tensor_mul(t2[:], raw[:], cos_sb[:, sl])
                    t1 = ropep.tile([128, 512], f32, tag="t1")
                    nc.vector.tensor_mul(t1[:], rp[:], sin_sb[:, sl])
                    nc.gpsimd.tensor_tensor(dest[:, sl], t1[:], t2[:], ALU.add)

                def emit_proj_group(g, ti, c, evac_eng):
                    sl = slice(512 * c, 512 * (c + 1))
                    coff = ti * HPG * D + 128 * g
                    pp = pjp.tile([128, 512], f32, tag="pj", bufs=3, name="pp")
                    for e in range(EK):
                        nc.tensor.matmul(
                            pp[:],
                            wqk_sb[e][:, coff : coff + 128],
                            xt_sb[e][:, sl],
                            start=(e == 0),
                            stop=(e == EK - 1),
                        )
                    raw = ropep.tile([128, 512], bf16, tag="raw", name="raw")
                    if evac_eng == "scalar":
                        nc.scalar.copy(raw[:], pp[:])
                    else:
                        nc.vector.tensor_copy(raw[:], pp[:])
                    dest = (qT, kT)[ti][g]
                    rope_pend.append((dest, sl, raw))
                    if len(rope_pend) > 1:
                        rope_tail()

                for g in range(PAIRS):
                    for ti in range(2):      # 0 = q, 1 = k
                        for c in range(4):
                            emit_proj_group(g, ti, c, ("scalar", "vector")[c % 2])
                while rope_pend:
                    rope_tail()

                # v projection: all 4 heads per sk-tile, evac to bf16 + aug
                for st in range(N_SK):
                    vp_ps = pjp.tile([128, HPG * D], f32, tag="pj", bufs=3, name="vps")
                    for e in range(EK):
                        nc.tensor.matmul(
                            vp_ps[:],
                            xt_sb[e][:, 128 * st : 128 * (st + 1)],
                            wv_sb[e][:],
                            start=(e == 0),
                            stop=(e == EK - 1),
                        )
                    if st % 2 == 0:
                        nc.vector.tensor_copy(
                            v_c[:, st, :, 0:64],
                            vp_ps.rearrange("p (h c) -> p h c", h=HPG),
                        )
                    else:
                        nc.scalar.copy(
                            v_c[:, st, :, 0:64],
                            vp_ps.rearrange("p (h c) -> p h c", h=HPG),
                        )

            if DBG:
                nc.sync.dma_start(d_qT[:], qT[0][:])
                nc.sync.dma_start(d_kT[:], kT[0][:])

            # ---------------- phase 2: attention, pairs interleaved ----------
            with tc.tile_pool(name="aps", bufs=1, space="PSUM") as psp:
                def attnv(g, sk, e_t, oT):
                    for h in range(2):
                        hh = 2 * g + h
                        nc.tensor.matmul(
                            oT[g][h][:],
                            v_c[:, sk, hh, 0:65],
                            e_t[:, h, :],
                            start=(sk == 0),
                            stop=(sk == N_SK - 1),
                        )

                for ch in range(N_CH):
                    csl = slice(SQ * ch, SQ * (ch + 1))
                    sps = psp.tile([128, 2, 2, SQ], f32, tag="sps", name="sps")
                    oT = [
                        [
                            psp.tile([65, SQ], f32, tag=f"oT{g}{h}", name=f"oT{g}{h}")
                            for h in range(2)
                        ]
                        for g in range(PAIRS)
                    ]
                    e_hist = {0: [], 1: []}
                    for sk in range(N_SK):
                        sksl = slice(128 * sk, 128 * (sk + 1))
                        for g in range(PAIRS):
                            for h in range(2):
                                pb = 64 * h
                                nc.tensor.matmul(
                                    sps[:, g, h, :],
                                    kT[g][pb : pb + 64, sksl],
                                    qT[g][pb : pb + 64, csl],
                                    start=True,
                                    stop=True,
                                )
                            e_cur = expp.tile(
                                [128, 2, SQ], bf16, tag=f"e{g}", bufs=3, name="ecur"
                            )
                            if sk % 4 == 3:
                                # Schraudolph fast-exp on DVE + gpsimd convert
                                ei = expp.tile(
                                    [128, 2, SQ], i32, tag=f"ei{g}", bufs=2, name="ei"
                                )
                                nc.vector.tensor_scalar(
                                    ei[:], sps[:, g, :, :], SCH_A, SCH_B,
                                    ALU.mult, ALU.add,
                                )
                                nc.gpsimd.tensor_scalar(
                                    e_cur[:], ei.bitcast(f32), 1.0, 0.0,
                                    ALU.mult, ALU.add,
                                )
                            else:
                                nc.scalar.activation(
                                    e_cur[:],
                                    sps[:, g, :, :],
                                    AF.Exp,
                                    scale=0.125,
                                    bias=bias_t[:],
                                )
                            e_hist[g].append(e_cur)
                            # attn@v lags one sk so its exp input is complete
                            if sk >= 1:
                                attnv(g, sk - 1, e_hist[g][sk - 1], oT)
                        heat(1)
                    for g in range(PAIRS):
                        attnv(g, N_SK - 1, e_hist[g][N_SK - 1], oT)

                    # ---- normalize: row 64 of oT holds the denominators ----
                    for g in range(PAIRS):
                        for h in range(2):
                            o_s = rcp.tile([65, SQ], bf16, tag=f"o{g}{h}", name="os")
                            nc.vector.tensor_copy(o_s[:], oT[g][h][:])
                            heat(2)
                            db = psp.tile([64, SQ], f32, tag=f"oT{g}{h}", name="db")
                            nc.tensor.matmul(
                                db[:], onesrow[64:65, :], o_s[64:65, :],
                                start=True, stop=True,
                            )
                            rb = rcp.tile([64, SQ], f32, tag=f"rb{g}{h}", name="rb")
                            nc.vector.reciprocal_approx_fast(rb[:], db[:])
                            if h == 0:
                                nc.vector.tensor_mul(
                                    att_o[g][0:64, csl], o_s[0:64, :], rb[:]
                                )
                            else:
                                aoB = rcp.tile([64, SQ], bf16, tag=f"aoB{g}", name="aoB")
                                nc.vector.tensor_mul(aoB[:], o_s[0:64, :], rb[:])
                                nc.sync.dma_start(att_o[g][64:128, csl], aoB[:])

            # ---------------- phase 3: output projection ----------------
            with tc.tile_pool(name="ops", bufs=1, space="PSUM") as opp:
                for st in range(S // 128):
                    ssl = slice(128 * st, 128 * (st + 1))
                    ot = osbp.tile([128, E], bf16, tag="ot")
                    for n in range(2):
                        nsl = slice(512 * n, 512 * (n + 1))
                        op = opp.tile([128, 512], f32, tag="op", bufs=3, name="op")
                        for g in range(PAIRS):
                            nc.tensor.matmul(
                                op[:],
                                att_o[g][:, ssl],
                                wo_sb[g][:, nsl],
                                start=(g == 0),
                                stop=(g == PAIRS - 1),
                            )
                        if n == 0:
                            nc.vector.tensor_copy(ot[:, nsl], op[:])
                        else:
                            nc.scalar.copy(ot[:, nsl], op[:])
                    nc.sync.dma_start(out[ssl, :], ot[:])

    nc.compile()
    return nc


def _get_program():
    if "nc" not in _BUILT:
        _BUILT["nc"] = _build_program()
    return _BUILT["nc"]


def _host_inputs(x, W_qkv, W_out):
    """Build the 8 per-core input maps."""
    import ml_dtypes

    f = np.float32
    bf = ml_dtypes.bfloat16
    x = np.asarray(x, dtype=f)
    W_qkv = np.asarray(W_qkv, dtype=f)
    W_out = np.asarray(W_out, dtype=f)

    inv_freq = 1.0 / (ROPE_THETA ** (np.arange(0, D, 2, dtype=np.float64) / D))
    p = np.arange(128)
    freq_row = inv_freq[(p % D) // 2]  # [128]
    ang = freq_row[:, None] * np.arange(S, dtype=np.float64)[None, :]  # [128, S]
    cos_t = np.cos(ang).astype(f)
    sign = np.where(p % 2 == 0, -1.0, 1.0)[:, None]
    sin_t = (np.sin(ang) * sign).astype(f)

    msw = np.zeros((128, 128), dtype=f)
    msw[p, p ^ 1] = 1.0

    maps = []
    for core in range(N_CORES):
        b, hg = divmod(core, HG)
        hs = [HPG * hg + i for i in range(HPG)]
        w_qk = np.concatenate(
            [W_qkv[:, h * D : (h + 1) * D] for h in hs]
            + [W_qkv[:, ATT + h * D : ATT + (h + 1) * D] for h in hs],
            axis=1,
        )
        w_v = np.concatenate(
            [W_qkv[:, 2 * ATT + h * D : 2 * ATT + (h + 1) * D] for h in hs], axis=1
        )
        w_o = np.concatenate([W_out[h * D : (h + 1) * D, :] for h in hs], axis=0)
        maps.append(
            {
                "xT": np.ascontiguousarray(x[b].T).astype(bf),
                "w_qk": np.ascontiguousarray(w_qk).astype(bf),
                "w_v": np.ascontiguousarray(w_v).astype(bf),
                "w_o": np.ascontiguousarray(w_o).astype(bf),
                "cos_t": cos_t,
                "sin_t": sin_t,
                "mswap": msw.astype(bf),
            }
        )
    return maps


def kernel(x, W_qkv, W_out):
    from concourse.bass_utils import run_bass_kernel_spmd

    nc = _get_program()
    maps = _host_inputs(x, W_qkv, W_out)
    res = run_bass_kernel_spmd(nc, maps, core_ids=list(range(N_CORES)))
    out = np.zeros((B, S, E), dtype=np.float32)
    for core in range(N_CORES):
        b = core // HG
        out[b] += np.asarray(res.results[core]["out"], dtype=np.float32)
    return out
